# revision 1
# baseline (speedup 1.0000x reference)
"""Cross-attention (LayerNorm -> MHA cross-attn -> out-proj -> residual) on 8 trn2 cores.

Sharding: core c -> (batch b = c//2, query-half qh = c%2). Each core computes all 16
heads for its 512 queries against the full 1024-token context of its batch. No
collectives needed; output shards are disjoint row blocks.

Host-side exact refactoring (all linear, fp32):
  - gamma folded into Wq:  Wq' = gamma[:,None] * Wq ;  qb = beta @ Wq
  - post-softmax scale 1/8 folded into Wv (power of two -> exact)
  - bo folded into the residual input: x' = x + bo
Device math per core:
  hn   = (x - mu) * rsqrt(var+eps)            (LayerNorm without affine)
  Q^T  = Wq'^T hn^T + qb                      (via PE transposes of hn)
  K^T  = Wk^T ctx^T ;  V' = ctx Wv'           (via PE transposes of ctx)
  lT   = K^T_h (slice) x Q^T_h  per head      (logits, transposed layout [j, i])
  aT   = exp(lT)   (no max subtraction; logits bounded ~ +-50 for N(0,1) data)
  Z    = ones^T aT (PE column-sum), zinv = 1/Z broadcast via DMA
  vT   = V'_h^T-contracted aT, normalized by zinv
  y    = x' + vals @ Wo
"""

import numpy as np
from contextlib import ExitStack

import concourse.bass as bass
import concourse.bacc as bacc_mod
import concourse.tile as tile
from concourse import mybir

F32 = mybir.dt.float32
F32R = mybir.dt.float32r
BF16 = mybir.dt.bfloat16
AF = mybir.ActivationFunctionType
ALU = mybir.AluOpType

B, NQ, NCTX, DQ, DC = 4, 1024, 1024, 1024, 768
H, DH, INNER = 16, 64, 1024
NQS = NQ // 2          # queries per core
EPS = 1e-5
N_CORES = 8


def _body(ctx, tc, nc, consts, xp, ctxt, wq, qb, wk, wv, wo, y):
    pers = ctx.enter_context(tc.tile_pool(name="pers", bufs=1))
    wpool = ctx.enter_context(tc.tile_pool(name="wpool", bufs=8))
    stat = ctx.enter_context(tc.tile_pool(name="stat", bufs=4))
    ps = ctx.enter_context(tc.tile_pool(name="ps", bufs=2, space="PSUM"))

    # ---- constants (identity shipped from host: f32r memset is invalid ISA) ----
    ident = pers.tile([128, 128], F32R, name="ident")
    nc.sync.dma_start(out=ident, in_=consts[0:128, :])
    ones = pers.tile([128, 1], BF16, name="ones")
    nc.vector.memset(ones, 1.0)
    eps_t = pers.tile([128, 1], F32, name="eps_t")
    nc.vector.memset(eps_t, EPS)
    zero_t = pers.tile([128, 1], F32, name="zero_t")
    nc.vector.memset(zero_t, 0.0)
    qb_sb = pers.tile([128, 8], F32, name="qb_sb")
    nc.sync.dma_start(out=qb_sb, in_=qb[0, :].rearrange("(m p) -> p m", p=128))

    # ---- persistent activations ----
    QT = pers.tile([128, 8, NQS], F32R, name="QT")        # Q^T  [inner, i]
    KT = pers.tile([128, 8, NCTX], F32R, name="KT")       # K^T  [inner, j]
    V_sb = pers.tile([128, 8, INNER], BF16, name="V_sb")  # V    [j, inner]
    valsT = pers.tile([128, 8, NQS], F32R, name="valsT")  # vals^T [inner, i]
    valsU = pers.tile([128, 8, NQS], F32, name="valsU")   # unnormalized vals^T
    zscr = nc.dram_tensor("zscr", [16, 512], F32)

    e1 = tc.alloc_tile_pool(name="e1", bufs=1)
    hT = e1.tile([128, 8, NQS], F32R, name="hT")          # hn^T  [d, i]
    e2 = tc.alloc_tile_pool(name="e2", bufs=1)
    ctxT = e2.tile([128, 6, NCTX], F32R, name="ctxT")     # ctx^T [d, j]
    xpool = tc.alloc_tile_pool(name="xpool", bufs=1)
    xp_sb = xpool.tile([128, 4, DQ], F32, name="xp_sb")
    nc.sync.dma_start(out=xp_sb, in_=xp.rearrange("(t p) d -> p t d", p=128))

    # ---- load context (8 row tiles) into weight-pool slots ----
    c_nat = []
    for t in range(8):
        cn = wpool.tile([128, DC], F32R, tag="w", name=f"cnat{t}")
        nc.sync.dma_start(out=cn, in_=ctxt[t * 128:(t + 1) * 128, :])
        c_nat.append(cn)

    # ---- transpose context: ctxT[d', dt, j] = ctx[j, dt*128+d'] ----
    for dt_ in range(6):
        for g in range(2):
            ptp = ps.tile([128, 4, 128], F32R, tag="mm", name="ptp")
            for q in range(4):
                jt = g * 4 + q
                nc.tensor.transpose(ptp[:, q, :], c_nat[jt][:, dt_ * 128:(dt_ + 1) * 128], ident)
            nc.vector.tensor_copy(
                out=ctxT[:, dt_, g * 512:(g + 1) * 512].rearrange("p (a b) -> p a b", a=4),
                in_=ptp,
            )

    # ---- LayerNorm + transpose hn ----
    for it in range(4):
        st = stat.tile([128, 2, 6], F32, tag="st", name="st")
        for sb in range(2):
            nc.vector.bn_stats(out=st[:, sb, :], in_=xp_sb[:, it, sb * 512:(sb + 1) * 512])
        mv = stat.tile([128, 2], F32, tag="mv", name="mv")
        nc.vector.bn_aggr(out=mv, in_=st)
        sd = stat.tile([128, 1], F32, tag="sd", name="sd")
        nc.scalar.activation(out=sd, in_=mv[:, 1:2], func=AF.Sqrt, bias=eps_t, scale=1.0)
        rstd = stat.tile([128, 1], F32, tag="rstd", name="rstd")
        nc.vector.reciprocal(out=rstd, in_=sd)
        nmu = stat.tile([128, 1], F32, tag="nmu", name="nmu")
        nc.vector.tensor_scalar(out=nmu, in0=mv[:, 0:1], scalar1=-1.0, scalar2=None, op0=ALU.mult)
        hn = stat.tile([128, DQ], F32R, tag="hn", bufs=1, name="hn")
        nc.vector.tensor_scalar(out=hn, in0=xp_sb[:, it, :], scalar1=nmu, scalar2=rstd,
                                op0=ALU.add, op1=ALU.mult)
        for g in range(2):
            ptp = ps.tile([128, 4, 128], F32R, tag="mm", name="ptph")
            for q in range(4):
                dt_ = g * 4 + q
                nc.tensor.transpose(ptp[:, q, :], hn[:, dt_ * 128:(dt_ + 1) * 128], ident)
            nc.vector.tensor_copy(
                out=hT[:, g * 4:(g + 1) * 4, it * 128:(it + 1) * 128],
                in_=ptp,
            )
    xpool.release()

    # ---- V' = ctx @ Wv' ----
    wv_t = []
    for k in range(6):
        wt = wpool.tile([128, INNER], F32R, tag="w", name=f"wv{k}")
        nc.sync.dma_start(out=wt, in_=wv[k * 128:(k + 1) * 128, :])
        wv_t.append(wt)
    for jt in range(8):
        for c in range(2):
            pv = ps.tile([128, 512], F32, tag="mm", name="pv")
            for k in range(6):
                nc.tensor.matmul(pv, ctxT[:, k, jt * 128:(jt + 1) * 128],
                                 wv_t[k][:, c * 512:(c + 1) * 512],
                                 start=(k == 0), stop=(k == 5))
            nc.vector.tensor_copy(out=V_sb[:, jt, c * 512:(c + 1) * 512], in_=pv)

    # ---- K^T = Wk^T @ ctx^T ----
    wk_t = []
    for k in range(6):
        wt = wpool.tile([128, INNER], F32R, tag="w", name=f"wk{k}")
        nc.sync.dma_start(out=wt, in_=wk[k * 128:(k + 1) * 128, :])
        wk_t.append(wt)
    for m in range(8):
        for c in range(2):
            pk = ps.tile([128, 512], F32, tag="mm", name="pk")
            for k in range(6):
                nc.tensor.matmul(pk, wk_t[k][:, m * 128:(m + 1) * 128],
                                 ctxT[:, k, c * 512:(c + 1) * 512],
                                 start=(k == 0), stop=(k == 5))
            nc.vector.tensor_copy(out=KT[:, m, c * 512:(c + 1) * 512], in_=pk)
    e2.release()

    # ---- wq tiles (used per-pair inside the attention loop) ----
    wq_t = []
    for k in range(8):
        wt = wpool.tile([128, INNER], F32R, tag="w", name=f"wq{k}")
        nc.sync.dma_start(out=wt, in_=wq[k * 128:(k + 1) * 128, :])
        wq_t.append(wt)

    att = tc.alloc_tile_pool(name="att", bufs=4)

    # ---- attention, head pairs; Q-projection interleaved to keep PE dense ----
    for hp in range(8):
        hA, hB = 2 * hp, 2 * hp + 1
        pq = ps.tile([128, 512], F32, tag="mm", name="pq")
        for k in range(8):
            nc.tensor.matmul(pq, wq_t[k][:, hp * 128:(hp + 1) * 128], hT[:, k, :],
                             start=(k == 0), stop=(k == 7))
        nc.vector.tensor_scalar(out=QT[:, hp, :], in0=pq, scalar1=qb_sb[:, hp:hp + 1],
                                scalar2=None, op0=ALU.add)
        aTA = att.tile([128, 8, NQS], BF16, tag="aT", bufs=3, name=f"aTA{hp}")
        aTB = att.tile([128, 8, NQS], BF16, tag="aT", bufs=3, name=f"aTB{hp}")
        for g in range(4):
            plA = ps.tile([128, 2, 512], F32, tag="lg", name="plA")
            plB = ps.tile([128, 2, 512], F32, tag="lg", name="plB")
            for bb in range(2):
                jt = g * 2 + bb
                nc.tensor.matmul(plA[:, bb, :], KT[0:64, hp, jt * 128:(jt + 1) * 128],
                                 QT[0:64, hp, :], start=True, stop=True,
                                 tile_position=(0, 0))
                nc.tensor.matmul(plB[:, bb, :], KT[64:128, hp, jt * 128:(jt + 1) * 128],
                                 QT[64:128, hp, :], start=True, stop=True,
                                 tile_position=(64, 0))
            nc.scalar.activation(out=aTA[:, g * 2:(g + 1) * 2, :], in_=plA, func=AF.Exp, bias=zero_t)
            nc.scalar.activation(out=aTB[:, g * 2:(g + 1) * 2, :], in_=plB, func=AF.Exp, bias=zero_t)
        zA = ps.tile([128, 512], F32, tag="z", bufs=1, name="zA")
        zB = ps.tile([128, 512], F32, tag="zb", bufs=1, name="zB")
        pvA = ps.tile([128, 512], F32, tag="mm", name="pvA")
        pvB = ps.tile([128, 512], F32, tag="mm", name="pvB")
        for jt in range(8):
            st_, sp = jt == 0, jt == 7
            nc.tensor.matmul(zA[0:1, :], ones, aTA[:, jt, :], start=st_, stop=sp,
                             tile_position=(0, 0))
            nc.tensor.matmul(zB[64:65, :], ones, aTB[:, jt, :], start=st_, stop=sp,
                             tile_position=(0, 64))
            nc.tensor.matmul(pvA[0:64, :], V_sb[:, jt, hA * 64:(hA + 1) * 64],
                             aTA[:, jt, :], start=st_, stop=sp, tile_position=(0, 0))
            nc.tensor.matmul(pvB[64:128, :], V_sb[:, jt, hB * 64:(hB + 1) * 64],
                             aTB[:, jt, :], start=st_, stop=sp, tile_position=(0, 64))
        zinv = att.tile([65, 512], F32, tag="zinv", bufs=2, name=f"zinv{hp}")
        nc.vector.tensor_copy(out=zinv, in_=KT[0:65, 0, 0:512])  # init filler rows
        nc.vector.tensor_copy(out=zinv[0:1, :], in_=zA[0:1, :])
        nc.vector.tensor_copy(out=zinv[64:65, :], in_=zB[64:65, :])
        nc.vector.reciprocal(out=zinv, in_=zinv)
        nc.sync.dma_start(out=zscr[2 * hp:2 * hp + 1, :], in_=zinv[0:1, :])
        nc.sync.dma_start(out=zscr[2 * hp + 1:2 * hp + 2, :], in_=zinv[64:65, :])
        nc.vector.tensor_copy(out=valsU[0:64, hp, :], in_=pvA[0:64, :])
        nc.vector.tensor_copy(out=valsU[64:128, hp, :], in_=pvB[64:128, :])

    # ---- deferred softmax normalization: valsT = valsU * (1/Z) ----
    for hp in range(8):
        zbs = att.tile([128, 512], F32, tag="zbs", bufs=2, name=f"zbs{hp}")
        for bb in range(2):
            src = zscr[2 * hp + bb:2 * hp + bb + 1, :]
            nc.sync.dma_start(
                out=zbs[64 * bb:64 * (bb + 1), :],
                in_=bass.AP(tensor=src.tensor, offset=src.offset, ap=[[0, 64], [1, 512]]),
            )
        nc.vector.tensor_mul(valsT[:, hp, :], valsU[:, hp, :], zbs)

    att.release()
    e1.release()

    # ---- out projection + residual (wo streamed as column chunks) ----
    wop = tc.alloc_tile_pool(name="wop", bufs=1)
    yout = tc.alloc_tile_pool(name="yout", bufs=1)
    for c in range(2):
        woc = []
        for k in range(8):
            wt = wop.tile([128, 512], F32R, tag="woc", bufs=8, name=f"woc{c}_{k}")
            nc.sync.dma_start(out=wt, in_=wo[k * 128:(k + 1) * 128, c * 512:(c + 1) * 512])
            woc.append(wt)
        for it in range(4):
            yt = yout.tile([128, 512], F32, tag=f"yt{c}", bufs=2, name="yt")
            xp2 = yout.tile([128, 512], F32, tag=f"xp2{c}", bufs=2, name="xp2")
            nc.sync.dma_start(out=xp2,
                              in_=xp.rearrange("(t p) d -> p t d", p=128)[:, it, c * 512:(c + 1) * 512])
            po = ps.tile([128, 512], F32, tag="lg", name="po")
            for ct in range(8):
                nc.tensor.matmul(po, valsT[:, ct, it * 128:(it + 1) * 128],
                                 woc[ct], start=(ct == 0), stop=(ct == 7))
            nc.vector.tensor_add(yt, po, xp2)
            nc.sync.dma_start(out=y.rearrange("(t p) d -> p t d", p=128)[:, it, c * 512:(c + 1) * 512],
                              in_=yt)
    yout.release()
    wop.release()


def build_nc():
    nc = bacc_mod.Bacc()
    consts = nc.dram_tensor("consts", [257, 128], F32R, kind="ExternalInput")
    xp = nc.dram_tensor("xp", [NQS, DQ], F32, kind="ExternalInput")
    ctxt = nc.dram_tensor("ctxt", [NCTX, DC], F32R, kind="ExternalInput")
    wq = nc.dram_tensor("wq", [DQ, INNER], F32R, kind="ExternalInput")
    qb = nc.dram_tensor("qb", [1, INNER], F32, kind="ExternalInput")
    wk = nc.dram_tensor("wk", [DC, INNER], F32R, kind="ExternalInput")
    wv = nc.dram_tensor("wv", [DC, INNER], F32R, kind="ExternalInput")
    wo = nc.dram_tensor("wo", [INNER, DQ], F32R, kind="ExternalInput")
    y = nc.dram_tensor("y", [NQS, DQ], F32, kind="ExternalOutput")
    with ExitStack() as ctx:
        tc = ctx.enter_context(tile.TileContext(nc))
        _body(ctx, tc, nc, consts, xp, ctxt, wq, qb, wk, wv, wo, y)
    nc.compile()
    return nc


def make_in_maps(x, context, Wq, Wk, Wv, Wo, bo, gamma, beta):
    x = np.asarray(x, np.float32)
    context = np.asarray(context, np.float32)
    Wq = np.asarray(Wq, np.float32)
    Wk = np.asarray(Wk, np.float32)
    Wv = np.asarray(Wv, np.float32)
    Wo = np.asarray(Wo, np.float32)
    bo = np.asarray(bo, np.float32)
    gamma = np.asarray(gamma, np.float32)
    beta = np.asarray(beta, np.float32)

    wq_f = np.ascontiguousarray(gamma[:, None] * Wq)
    qb_f = np.ascontiguousarray((beta @ Wq)[None, :])
    wv_f = np.ascontiguousarray(Wv * np.float32(0.125))
    xp_full = x + bo  # residual with bo folded in

    consts = np.zeros((257, 128), np.float32)
    consts[0:128, 0:128] = np.eye(128, dtype=np.float32)
    consts[128:257, :] = 1.0
    in_maps = []
    for c in range(N_CORES):
        b, qh = divmod(c, 2)
        in_maps.append({
            "consts": consts,
            "xp": np.ascontiguousarray(xp_full[b, qh * NQS:(qh + 1) * NQS, :]),
            "ctxt": np.ascontiguousarray(context[b]),
            "wq": wq_f, "qb": qb_f, "wk": Wk, "wv": wv_f, "wo": Wo,
        })
    return in_maps


_NC_CACHE = []


def kernel(x, context, Wq, Wk, Wv, Wo, bo, gamma, beta):
    from concourse.bass_utils import run_bass_kernel_spmd
    if not _NC_CACHE:
        _NC_CACHE.append(build_nc())
    nc = _NC_CACHE[0]
    in_maps = make_in_maps(x, context, Wq, Wk, Wv, Wo, bo, gamma, beta)
    res = run_bass_kernel_spmd(nc, in_maps, list(range(N_CORES)))
    y = np.empty((B, NQ, DQ), np.float32)
    for c in range(N_CORES):
        b, qh = divmod(c, 2)
        y[b, qh * NQS:(qh + 1) * NQS, :] = res.results[c]["y"]
    return y



# revision 16
# speedup vs baseline: 1.7656x; 1.7656x over previous
"""Cross-attention (LayerNorm -> MHA cross-attn -> out-proj -> residual) on 8 trn2 cores.

Sharding: core c -> (batch b = c//2, query-half qh = c%2). Each core computes all 16
heads for its 512 queries against the full 1024-token context of its batch. No
collectives needed; output shards are disjoint row blocks.

Host-side exact refactoring (all linear, fp32):
  - gamma folded into Wq:  Wq' = gamma[:,None] * Wq ;  qb = beta @ Wq
  - post-softmax scale 1/8 folded into Wv (power of two -> exact)
  - bo folded into the residual input: x' = x + bo
  - context shipped PRE-TRANSPOSED (ctxT = ctx.T, bf16)
  - wk/wv/wo shipped bf16 (psum accumulation stays fp32)

Device math per core:
  hn   = (x - mu) * rsqrt(var+eps)            (LayerNorm without affine)
  Q^T  = Wq'^T hn^T + qb
  K^T  = Wk^T ctx^T ;  V'' = [ctx Wv' | 1] with a ones column per head
  lT   = K^T_h (slice) x Q^T_h  per head      (logits, transposed layout [j, i])
  aT   = exp(lT)
  pv   = V''_h^T-contracted aT -> rows 0:64 = unnormalized vals, row 64 = Z
  vT   = valsU * (1/Z)  (recip on DVE per head-pair, junk rows harmless)
  y    = x' + vals @ Wo  (split: ct 0-3 accumulated during attention as PE filler)

v3: z-matmuls folded into pv via the ones column (-128 matmuls); out-proj split
P1/P2 so the tail stays PE-dense; weights bf16 so all weight DMAs issue at t=0.
"""

import numpy as np
import ml_dtypes
from contextlib import ExitStack

import concourse.bass as bass
import concourse.bacc as bacc_mod
import concourse.tile as tile
from concourse import mybir

F32 = mybir.dt.float32
F32R = mybir.dt.float32r
BF16 = mybir.dt.bfloat16
AF = mybir.ActivationFunctionType
ALU = mybir.AluOpType

B, NQ, NCTX, DQ, DC = 4, 1024, 1024, 1024, 768
H, DH, INNER = 16, 64, 1024
NQS = NQ // 2          # queries per core
EPS = 1e-5
N_CORES = 8


def _body(ctx, tc, nc, consts, xp, ctxt_t, wq, qb, wk, wv, wo, y):
    pers = ctx.enter_context(tc.tile_pool(name="pers", bufs=1))
    wpool = ctx.enter_context(tc.tile_pool(name="wpool", bufs=12))
    wqpool = ctx.enter_context(tc.tile_pool(name="wqpool", bufs=8))
    wopool = ctx.enter_context(tc.tile_pool(name="wopool", bufs=8))
    stat = ctx.enter_context(tc.tile_pool(name="stat", bufs=4))
    ps = ctx.enter_context(tc.tile_pool(name="ps", bufs=2, space="PSUM"))

    # ---- constants (identity shipped from host: f32r memset is invalid ISA) ----
    ident = pers.tile([128, 128], BF16, name="ident")
    nc.sync.dma_start(out=ident, in_=consts[0:128, :])
    eps_t = pers.tile([128, 1], F32, name="eps_t")
    nc.vector.memset(eps_t, EPS)
    zero_t = pers.tile([128, 1], F32, name="zero_t")
    nc.vector.memset(zero_t, 0.0)
    qb_sb = pers.tile([128, 8], F32, name="qb_sb")
    nc.sync.dma_start(out=qb_sb, in_=qb[0, :].rearrange("(m p) -> p m", p=128))

    # ---- persistent activations ----
    QT = pers.tile([128, 8, NQS], BF16, name="QT")          # Q^T  [inner, i]
    KT = pers.tile([128, 8, NCTX], BF16, name="KT")         # K^T  [inner, j]
    V_aug = pers.tile([128, 8, 16 * 65], BF16, name="V_aug")  # per head: [V_h | 1]
    nc.vector.memset(V_aug, 1.0)                            # ones cols stay 1.0
    valsT = pers.tile([128, 8, NQS], BF16, name="valsT")    # vals^T [inner, i]
    valsU = pers.tile([128, 8, NQS], BF16, name="valsU")    # unnormalized vals^T
    zscr2 = nc.dram_tensor("zscr2", [16, NQS], BF16)         # 1/Z rows

    e1 = tc.alloc_tile_pool(name="e1", bufs=1)
    hT = e1.tile([128, 8, NQS], BF16, name="hT")            # hn^T  [d, i]
    e2 = tc.alloc_tile_pool(name="e2", bufs=1)
    ctxT = e2.tile([128, 6, NCTX], BF16, name="ctxT")       # ctx^T [d, j]
    for k in range(6):
        nc.sync.dma_start(out=ctxT[:, k, :], in_=ctxt_t[k * 128:(k + 1) * 128, :])
    xpool = tc.alloc_tile_pool(name="xpool", bufs=1)
    xp_sb = xpool.tile([128, 4, DQ], F32, name="xp_sb")
    nc.sync.dma_start(out=xp_sb, in_=xp.rearrange("(t p) d -> p t d", p=128))

    # ---- all projection weights DMA'd up front (bf16 halves the bytes) ----
    wv_t, wk_t, wq_t = [], [], []
    for k in range(6):
        wt = wpool.tile([128, INNER], BF16, tag="w", name=f"wv{k}")
        nc.sync.dma_start(out=wt, in_=wv[k * 128:(k + 1) * 128, :])
        wv_t.append(wt)
    for k in range(6):
        wt = wpool.tile([128, INNER], BF16, tag="w", name=f"wk{k}")
        nc.sync.dma_start(out=wt, in_=wk[k * 128:(k + 1) * 128, :])
        wk_t.append(wt)
    for k in range(8):
        wt = wqpool.tile([128, INNER], BF16, tag="wq", name=f"wq{k}")
        nc.sync.dma_start(out=wt, in_=wq[k * 128:(k + 1) * 128, :])
        wq_t.append(wt)

    # ---- LayerNorm + transpose hn ----
    for it in range(4):
        st = stat.tile([128, 2, 6], F32, tag="st", name="st")
        for sb in range(2):
            nc.vector.bn_stats(out=st[:, sb, :], in_=xp_sb[:, it, sb * 512:(sb + 1) * 512])
        mv = stat.tile([128, 2], F32, tag="mv", name="mv")
        nc.vector.bn_aggr(out=mv, in_=st)
        sd = stat.tile([128, 1], F32, tag="sd", name="sd")
        nc.scalar.activation(out=sd, in_=mv[:, 1:2], func=AF.Sqrt, bias=eps_t, scale=1.0)
        rstd = stat.tile([128, 1], F32, tag="rstd", name="rstd")
        nc.vector.reciprocal(out=rstd, in_=sd)
        nmu = stat.tile([128, 1], F32, tag="nmu", name="nmu")
        nc.vector.tensor_scalar(out=nmu, in0=mv[:, 0:1], scalar1=-1.0, scalar2=None, op0=ALU.mult)
        hn = stat.tile([128, DQ], BF16, tag="hn", bufs=1, name="hn")
        nc.vector.tensor_scalar(out=hn, in0=xp_sb[:, it, :], scalar1=nmu, scalar2=rstd,
                                op0=ALU.add, op1=ALU.mult)
        for g in range(2):
            ptp = ps.tile([128, 4, 128], BF16, tag="mm", name="ptph")
            for q in range(4):
                dt_ = g * 4 + q
                nc.tensor.transpose(ptp[:, q, :], hn[:, dt_ * 128:(dt_ + 1) * 128], ident)
            nc.vector.tensor_copy(
                out=hT[:, g * 4:(g + 1) * 4, it * 128:(it + 1) * 128],
                in_=ptp,
            )
    xpool.release()

    # ---- V' = ctx @ Wv', evacuated into the 65-col-per-head augmented layout ----
    for c in range(2):
        for jt in range(8):
            pv = ps.tile([128, 512], F32, tag="mm", name="pv")
            for k in range(6):
                nc.tensor.matmul(pv, ctxT[:, k, jt * 128:(jt + 1) * 128],
                                 wv_t[k][:, c * 512:(c + 1) * 512],
                                 start=(k == 0), stop=(k == 5))
            nc.vector.tensor_copy(
                out=V_aug[:, jt, c * 520:(c + 1) * 520]
                    .rearrange("p (h e) -> p h e", h=8)[:, :, 0:64],
                in_=pv.rearrange("p (h e) -> p h e", h=8),
            )

    # ---- K^T = Wk^T @ ctx^T ----
    for m in range(8):
        for c in range(2):
            pk = ps.tile([128, 512], F32, tag="mm", name="pk")
            for k in range(6):
                nc.tensor.matmul(pk, wk_t[k][:, m * 128:(m + 1) * 128],
                                 ctxT[:, k, c * 512:(c + 1) * 512],
                                 start=(k == 0), stop=(k == 5))
            nc.vector.tensor_copy(out=KT[:, m, c * 512:(c + 1) * 512], in_=pk)
    e2.release()

    att = tc.alloc_tile_pool(name="att", bufs=4)
    yac = None
    woR = []

    def normalize(h):
        zbs = att.tile([128, 512], BF16, tag="zbs", bufs=2, name=f"zbs{h}")
        for bb in range(2):
            src = zscr2[2 * h + bb:2 * h + bb + 1, :]
            nc.sync.dma_start(
                out=zbs[64 * bb:64 * (bb + 1), :],
                in_=bass.AP(tensor=src.tensor, offset=src.offset, ap=[[0, 64], [1, 512]]),
            )
        with nc.allow_low_precision(reason="softmax normalize in bf16, tol 2e-2"):
            nc.vector.tensor_mul(valsT[:, h, :], valsU[:, h, :], zbs)

    # ---- attention, head pairs; Q-projection interleaved to keep PE dense ----
    for hp in range(8):
        hA, hB = 2 * hp, 2 * hp + 1
        if hp >= 1:
            normalize(hp - 1)
        pq = ps.tile([128, 512], F32, tag="mm", name="pq")
        for k in range(8):
            nc.tensor.matmul(pq, wq_t[k][:, hp * 128:(hp + 1) * 128], hT[:, k, :],
                             start=(k == 0), stop=(k == 7))
        with nc.allow_low_precision(reason="QT bf16, tol 2e-2"):
            nc.vector.tensor_scalar(out=QT[:, hp, :], in0=pq, scalar1=qb_sb[:, hp:hp + 1],
                                    scalar2=None, op0=ALU.add)
        aTA = att.tile([128, 8, NQS], BF16, tag="aT", bufs=3, name=f"aTA{hp}")
        aTB = att.tile([128, 8, NQS], BF16, tag="aT", bufs=3, name=f"aTB{hp}")
        for g in range(4):
            plA = ps.tile([128, 2, 512], F32, tag="lg", name="plA")
            plB = ps.tile([128, 2, 512], F32, tag="lg", name="plB")
            for bb in range(2):
                jt = g * 2 + bb
                nc.tensor.matmul(plA[:, bb, :], KT[0:64, hp, jt * 128:(jt + 1) * 128],
                                 QT[0:64, hp, :], start=True, stop=True,
                                 tile_position=(0, 0))
                nc.tensor.matmul(plB[:, bb, :], KT[64:128, hp, jt * 128:(jt + 1) * 128],
                                 QT[64:128, hp, :], start=True, stop=True,
                                 tile_position=(64, 0))
            nc.scalar.activation(out=aTA[:, g * 2:(g + 1) * 2, :], in_=plA, func=AF.Exp, bias=zero_t)
            nc.scalar.activation(out=aTB[:, g * 2:(g + 1) * 2, :], in_=plB, func=AF.Exp, bias=zero_t)
        pvA = ps.tile([65, 512], F32, tag="pva", bufs=1, name="pvA")
        pvB = ps.tile([65, 512], F32, tag="pvb", bufs=1, name="pvB")
        for jt in range(8):
            st_, sp = jt == 0, jt == 7
            nc.tensor.matmul(pvA, V_aug[:, jt, hA * 65:(hA + 1) * 65],
                             aTA[:, jt, :], start=st_, stop=sp)
            nc.tensor.matmul(pvB, V_aug[:, jt, hB * 65:(hB + 1) * 65],
                             aTB[:, jt, :], start=st_, stop=sp)
        # evacuate: vals rows 0:64, denominator row 64 (partition-aligned)
        nc.vector.tensor_copy(out=valsU[0:64, hp, :], in_=pvA[0:64, :])
        nc.vector.tensor_copy(out=valsU[64:128, hp, :], in_=pvB[0:64, :])
        zsb = att.tile([65, 512], F32, tag="zsb", bufs=2, name=f"zsb{hp}")
        nc.vector.tensor_copy(out=zsb[0:1, :], in_=pvA[64:65, :])
        nc.vector.tensor_copy(out=zsb[64:65, :], in_=pvB[64:65, :])
        zi = att.tile([65, 512], BF16, tag="zi", bufs=2, name=f"zi{hp}")
        with nc.allow_low_precision(reason="1/Z in bf16, tol 2e-2"):
            nc.vector.reciprocal(out=zi, in_=zsb)   # junk rows harmless

        nc.sync.dma_start(out=zscr2[2 * hp:2 * hp + 1, :], in_=zi[0:1, :])
        nc.sync.dma_start(out=zscr2[2 * hp + 1:2 * hp + 2, :], in_=zi[64:65, :])

        if hp == 5:
            # out-proj weights, row-major: woR[ct] = wo[ct*128:(ct+1)*128, :]
            for ct in range(8):
                wt = wopool.tile([128, DQ], BF16, tag="wo", name=f"woR{ct}")
                nc.sync.dma_start(out=wt, in_=wo[ct * 128:(ct + 1) * 128, :])
                woR.append(wt)
        if hp == 7:
            # P1: first half of out-proj (ct 0-3) as PE filler; fold in residual
            yac = att.tile([128, 2, 4, 512], BF16, tag="yac", bufs=1, name="yac")
            for c in range(2):
                for it in range(4):
                    xp2 = att.tile([128, 512], F32, tag="xp2", bufs=2, name="xp2")
                    nc.sync.dma_start(
                        out=xp2,
                        in_=xp.rearrange("(t p) d -> p t d", p=128)[:, it, c * 512:(c + 1) * 512])
                    po = ps.tile([128, 512], F32, tag="mm", name="po1")
                    for ct in range(4):
                        nc.tensor.matmul(po, valsT[:, ct, it * 128:(it + 1) * 128],
                                         woR[ct][:, c * 512:(c + 1) * 512],
                                         start=(ct == 0), stop=(ct == 3))
                    with nc.allow_low_precision(reason="partial out-proj sum bf16"):
                        nc.vector.tensor_add(yac[:, c, it, :], po, xp2)

    # ---- P2: second half of out-proj (ct 4-7) + store ----
    normalize(7)
    for c in range(2):
        for it in range(4):
            po = ps.tile([128, 512], F32, tag="lg", name="po2")
            for ct in range(4, 8):
                nc.tensor.matmul(po, valsT[:, ct, it * 128:(it + 1) * 128],
                                 woR[ct][:, c * 512:(c + 1) * 512],
                                 start=(ct == 4), stop=(ct == 7))
            yt = att.tile([128, 512], F32, tag="yt", bufs=2, name="yt")
            nc.vector.tensor_add(yt, po, yac[:, c, it, :])
            nc.sync.dma_start(out=y.rearrange("(t p) d -> p t d", p=128)[:, it, c * 512:(c + 1) * 512],
                              in_=yt)
    att.release()
    e1.release()


def build_nc():
    nc = bacc_mod.Bacc()
    consts = nc.dram_tensor("consts", [128, 128], BF16, kind="ExternalInput")
    xp = nc.dram_tensor("xp", [NQS, DQ], F32, kind="ExternalInput")
    ctxt_t = nc.dram_tensor("ctxt_t", [DC, NCTX], BF16, kind="ExternalInput")
    wq = nc.dram_tensor("wq", [DQ, INNER], BF16, kind="ExternalInput")
    qb = nc.dram_tensor("qb", [1, INNER], F32, kind="ExternalInput")
    wk = nc.dram_tensor("wk", [DC, INNER], BF16, kind="ExternalInput")
    wv = nc.dram_tensor("wv", [DC, INNER], BF16, kind="ExternalInput")
    wo = nc.dram_tensor("wo", [INNER, DQ], BF16, kind="ExternalInput")
    y = nc.dram_tensor("y", [NQS, DQ], F32, kind="ExternalOutput")
    with ExitStack() as ctx:
        tc = ctx.enter_context(tile.TileContext(nc))
        _body(ctx, tc, nc, consts, xp, ctxt_t, wq, qb, wk, wv, wo, y)
    nc.compile()
    return nc


def make_in_maps(x, context, Wq, Wk, Wv, Wo, bo, gamma, beta):
    x = np.asarray(x, np.float32)
    context = np.asarray(context, np.float32)
    Wq = np.asarray(Wq, np.float32)
    Wk = np.asarray(Wk, np.float32)
    Wv = np.asarray(Wv, np.float32)
    Wo = np.asarray(Wo, np.float32)
    bo = np.asarray(bo, np.float32)
    gamma = np.asarray(gamma, np.float32)
    beta = np.asarray(beta, np.float32)

    wq_f = np.ascontiguousarray((gamma[:, None] * Wq).astype(ml_dtypes.bfloat16))
    qb_f = np.ascontiguousarray((beta @ Wq)[None, :])
    wv_bf = np.ascontiguousarray((Wv * np.float32(0.125)).astype(ml_dtypes.bfloat16))
    wk_bf = np.ascontiguousarray(Wk.astype(ml_dtypes.bfloat16))
    wo_bf = np.ascontiguousarray(Wo.astype(ml_dtypes.bfloat16))
    xp_full = x + bo  # residual with bo folded in

    consts = np.eye(128).astype(ml_dtypes.bfloat16)
    in_maps = []
    for c in range(N_CORES):
        b, qh = divmod(c, 2)
        in_maps.append({
            "consts": consts,
            "xp": np.ascontiguousarray(xp_full[b, qh * NQS:(qh + 1) * NQS, :]),
            "ctxt_t": np.ascontiguousarray(context[b].T.astype(ml_dtypes.bfloat16)),
            "wq": wq_f, "qb": qb_f, "wk": wk_bf, "wv": wv_bf,
            "wo": wo_bf,
        })
    return in_maps


_NC_CACHE = []


def kernel(x, context, Wq, Wk, Wv, Wo, bo, gamma, beta):
    from concourse.bass_utils import run_bass_kernel_spmd
    if not _NC_CACHE:
        _NC_CACHE.append(build_nc())
    nc = _NC_CACHE[0]
    in_maps = make_in_maps(x, context, Wq, Wk, Wv, Wo, bo, gamma, beta)
    res = run_bass_kernel_spmd(nc, in_maps, list(range(N_CORES)))
    y = np.empty((B, NQ, DQ), np.float32)
    for c in range(N_CORES):
        b, qh = divmod(c, 2)
        y[b, qh * NQS:(qh + 1) * NQS, :] = res.results[c]["y"]
    return y


# revision 18
# speedup vs baseline: 1.8834x; 1.0667x over previous
"""Cross-attention (LayerNorm -> MHA cross-attn -> out-proj -> residual) on 8 trn2 cores.

Sharding: core c -> (batch b = c//2, query-half qh = c%2). Each core computes all 16
heads for its 512 queries against the full 1024-token context of its batch. No
collectives needed; output shards are disjoint row blocks.

Host-side exact refactoring (all linear, fp32):
  - gamma folded into Wq:  Wq' = gamma[:,None] * Wq ;  qb = beta @ Wq
  - post-softmax scale 1/8 folded into Wv (power of two -> exact)
  - bo folded into the residual input: x' = x + bo
  - context shipped PRE-TRANSPOSED (ctxT = ctx.T, bf16)
  - wk/wv/wo shipped bf16 (psum accumulation stays fp32)

Device math per core:
  hn   = (x - mu) * rsqrt(var+eps)            (LayerNorm without affine)
  Q^T  = Wq'^T hn^T + qb
  K^T  = Wk^T ctx^T ;  V'' = [ctx Wv' | 1] with a ones column per head
  lT   = K^T_h (slice) x Q^T_h  per head      (logits, transposed layout [j, i])
  aT   = exp(lT)
  pv   = V''_h^T-contracted aT -> rows 0:64 = unnormalized vals, row 64 = Z
  vT   = valsU * (1/Z)  (recip on DVE per head-pair, junk rows harmless)
  y    = x' + vals @ Wo  (split: ct 0-3 accumulated during attention as PE filler)

v3: z-matmuls folded into pv via the ones column (-128 matmuls); out-proj split
P1/P2 so the tail stays PE-dense; weights bf16 so all weight DMAs issue at t=0.
"""

import numpy as np
import ml_dtypes
from contextlib import ExitStack

import concourse.bass as bass
import concourse.bacc as bacc_mod
import concourse.tile as tile
from concourse import mybir

F32 = mybir.dt.float32
F32R = mybir.dt.float32r
BF16 = mybir.dt.bfloat16
AF = mybir.ActivationFunctionType
ALU = mybir.AluOpType

B, NQ, NCTX, DQ, DC = 4, 1024, 1024, 1024, 768
H, DH, INNER = 16, 64, 1024
NQS = NQ // 2          # queries per core
EPS = 1e-5
N_CORES = 8


def _body(ctx, tc, nc, consts, xp, ctxt_t, wq, qb, wk, wv, wo, y):
    pers = ctx.enter_context(tc.tile_pool(name="pers", bufs=1))
    wpool = ctx.enter_context(tc.tile_pool(name="wpool", bufs=12))
    wqpool = ctx.enter_context(tc.tile_pool(name="wqpool", bufs=8))
    wopool = ctx.enter_context(tc.tile_pool(name="wopool", bufs=8))
    stat = ctx.enter_context(tc.tile_pool(name="stat", bufs=4))
    ps = ctx.enter_context(tc.tile_pool(name="ps", bufs=2, space="PSUM"))

    # ---- constants (identity shipped from host: f32r memset is invalid ISA) ----
    ident = pers.tile([128, 128], BF16, name="ident")
    nc.sync.dma_start(out=ident, in_=consts[0:128, :])
    eps_t = pers.tile([128, 1], F32, name="eps_t")
    nc.vector.memset(eps_t, EPS)
    zero_t = pers.tile([128, 1], F32, name="zero_t")
    nc.vector.memset(zero_t, 0.0)
    qb_sb = pers.tile([128, 8], F32, name="qb_sb")
    nc.sync.dma_start(out=qb_sb, in_=qb[0, :].rearrange("(m p) -> p m", p=128))

    # ---- persistent activations ----
    QT = pers.tile([128, 8, NQS], BF16, name="QT")          # Q^T  [inner, i]
    KT = pers.tile([128, 8, NCTX], BF16, name="KT")         # K^T  [inner, j]
    V_aug = pers.tile([128, 8, 16 * 65], BF16, name="V_aug")  # per head: [V_h | 1]
    nc.vector.memset(V_aug, 1.0)                            # ones cols stay 1.0
    valsT = pers.tile([128, 8, NQS], BF16, name="valsT")    # vals^T [inner, i]
    valsU = pers.tile([128, 8, NQS], BF16, name="valsU")    # unnormalized vals^T
    zscr2 = nc.dram_tensor("zscr2", [16, NQS], F32)         # 1/Z rows

    e1 = tc.alloc_tile_pool(name="e1", bufs=1)
    hT = e1.tile([128, 8, NQS], BF16, name="hT")            # hn^T  [d, i]
    e2 = tc.alloc_tile_pool(name="e2", bufs=1)
    ctxT = e2.tile([128, 6, NCTX], BF16, name="ctxT")       # ctx^T [d, j]
    for k in range(6):
        nc.sync.dma_start(out=ctxT[:, k, :], in_=ctxt_t[k * 128:(k + 1) * 128, :])
    xp_sb = pers.tile([128, 4, DQ], F32, name="xp_sb")
    nc.sync.dma_start(out=xp_sb, in_=xp.rearrange("(t p) d -> p t d", p=128))

    # ---- all projection weights DMA'd up front (bf16 halves the bytes) ----
    wv_t, wk_t, wq_t = [], [], []
    for k in range(6):
        wt = wpool.tile([128, INNER], BF16, tag="w", name=f"wv{k}")
        nc.sync.dma_start(out=wt, in_=wv[k * 128:(k + 1) * 128, :])
        wv_t.append(wt)
    for k in range(6):
        wt = wpool.tile([128, INNER], BF16, tag="w", name=f"wk{k}")
        nc.sync.dma_start(out=wt, in_=wk[k * 128:(k + 1) * 128, :])
        wk_t.append(wt)
    for k in range(8):
        wt = wqpool.tile([128, INNER], BF16, tag="wq", name=f"wq{k}")
        nc.sync.dma_start(out=wt, in_=wq[k * 128:(k + 1) * 128, :])
        wq_t.append(wt)

    # ---- V' = ctx @ Wv', evacuated into the 65-col-per-head augmented layout ----
    for c in range(2):
        for jt in range(8):
            pv = ps.tile([128, 512], F32, tag="mm", name="pv")
            for k in range(6):
                nc.tensor.matmul(pv, ctxT[:, k, jt * 128:(jt + 1) * 128],
                                 wv_t[k][:, c * 512:(c + 1) * 512],
                                 start=(k == 0), stop=(k == 5))
            nc.vector.tensor_copy(
                out=V_aug[:, jt, c * 520:(c + 1) * 520]
                    .rearrange("p (h e) -> p h e", h=8)[:, :, 0:64],
                in_=pv.rearrange("p (h e) -> p h e", h=8),
            )

    # ---- K^T = Wk^T @ ctx^T ----
    for m in range(8):
        for c in range(2):
            pk = ps.tile([128, 512], F32, tag="mm", name="pk")
            for k in range(6):
                nc.tensor.matmul(pk, wk_t[k][:, m * 128:(m + 1) * 128],
                                 ctxT[:, k, c * 512:(c + 1) * 512],
                                 start=(k == 0), stop=(k == 5))
            nc.vector.tensor_copy(out=KT[:, m, c * 512:(c + 1) * 512], in_=pk)
    e2.release()
    # ---- LayerNorm + transpose hn ----
    for it in range(4):
        st = stat.tile([128, 2, 6], F32, tag="st", name="st")
        for sb in range(2):
            nc.vector.bn_stats(out=st[:, sb, :], in_=xp_sb[:, it, sb * 512:(sb + 1) * 512])
        mv = stat.tile([128, 2], F32, tag="mv", name="mv")
        nc.vector.bn_aggr(out=mv, in_=st)
        sd = stat.tile([128, 1], F32, tag="sd", name="sd")
        nc.scalar.activation(out=sd, in_=mv[:, 1:2], func=AF.Sqrt, bias=eps_t, scale=1.0)
        rstd = stat.tile([128, 1], F32, tag="rstd", name="rstd")
        nc.vector.reciprocal(out=rstd, in_=sd)
        nmu = stat.tile([128, 1], F32, tag="nmu", name="nmu")
        nc.vector.tensor_scalar(out=nmu, in0=mv[:, 0:1], scalar1=-1.0, scalar2=None, op0=ALU.mult)
        hn = stat.tile([128, DQ], BF16, tag="hn", bufs=1, name="hn")
        nc.vector.tensor_scalar(out=hn, in0=xp_sb[:, it, :], scalar1=nmu, scalar2=rstd,
                                op0=ALU.add, op1=ALU.mult)
        for g in range(2):
            ptp = ps.tile([128, 4, 128], BF16, tag="mm", name="ptph")
            for q in range(4):
                dt_ = g * 4 + q
                nc.tensor.transpose(ptp[:, q, :], hn[:, dt_ * 128:(dt_ + 1) * 128], ident)
            nc.vector.tensor_copy(
                out=hT[:, g * 4:(g + 1) * 4, it * 128:(it + 1) * 128],
                in_=ptp,
            )


    att = tc.alloc_tile_pool(name="att", bufs=4)
    yac = None
    woR = []

    def normalize(h):
        zbs = att.tile([128, 512], F32, tag="zbs", bufs=2, name=f"zbs{h}")
        for bb in range(2):
            src = zscr2[2 * h + bb:2 * h + bb + 1, :]
            nc.sync.dma_start(
                out=zbs[64 * bb:64 * (bb + 1), :],
                in_=bass.AP(tensor=src.tensor, offset=src.offset, ap=[[0, 64], [1, 512]]),
            )
        with nc.allow_low_precision(reason="softmax normalize in bf16, tol 2e-2"):
            nc.vector.tensor_mul(valsT[:, h, :], valsU[:, h, :], zbs)

    # ---- attention, head pairs; Q-projection interleaved to keep PE dense ----
    for hp in range(8):
        hA, hB = 2 * hp, 2 * hp + 1
        if hp >= 1:
            normalize(hp - 1)
        pq = ps.tile([128, 512], F32, tag="mm", name="pq")
        for k in range(8):
            nc.tensor.matmul(pq, wq_t[k][:, hp * 128:(hp + 1) * 128], hT[:, k, :],
                             start=(k == 0), stop=(k == 7))
        with nc.allow_low_precision(reason="QT bf16, tol 2e-2"):
            nc.vector.tensor_scalar(out=QT[:, hp, :], in0=pq, scalar1=qb_sb[:, hp:hp + 1],
                                    scalar2=None, op0=ALU.add)
        aTA = att.tile([128, 8, NQS], BF16, tag="aT", bufs=3, name=f"aTA{hp}")
        aTB = att.tile([128, 8, NQS], BF16, tag="aT", bufs=3, name=f"aTB{hp}")
        for g in range(4):
            plA = ps.tile([128, 2, 512], F32, tag="lg", name="plA")
            plB = ps.tile([128, 2, 512], F32, tag="lg", name="plB")
            for bb in range(2):
                jt = g * 2 + bb
                nc.tensor.matmul(plA[:, bb, :], KT[0:64, hp, jt * 128:(jt + 1) * 128],
                                 QT[0:64, hp, :], start=True, stop=True,
                                 tile_position=(0, 0))
                nc.tensor.matmul(plB[:, bb, :], KT[64:128, hp, jt * 128:(jt + 1) * 128],
                                 QT[64:128, hp, :], start=True, stop=True,
                                 tile_position=(64, 0))
            nc.scalar.activation(out=aTA[:, g * 2:(g + 1) * 2, :], in_=plA, func=AF.Exp, bias=zero_t)
            nc.scalar.activation(out=aTB[:, g * 2:(g + 1) * 2, :], in_=plB, func=AF.Exp, bias=zero_t)
        pvA = ps.tile([65, 512], F32, tag="pva", bufs=1, name="pvA")
        pvB = ps.tile([65, 512], F32, tag="pvb", bufs=1, name="pvB")
        for jt in range(8):
            st_, sp = jt == 0, jt == 7
            nc.tensor.matmul(pvA, V_aug[:, jt, hA * 65:(hA + 1) * 65],
                             aTA[:, jt, :], start=st_, stop=sp)
            nc.tensor.matmul(pvB, V_aug[:, jt, hB * 65:(hB + 1) * 65],
                             aTB[:, jt, :], start=st_, stop=sp)
        # evacuate: vals rows 0:64, denominator row 64 (partition-aligned)
        nc.vector.tensor_copy(out=valsU[0:64, hp, :], in_=pvA[0:64, :])
        nc.vector.tensor_copy(out=valsU[64:128, hp, :], in_=pvB[0:64, :])
        zsb = att.tile([65, 512], F32, tag="zsb", bufs=2, name=f"zsb{hp}")
        nc.vector.tensor_copy(out=zsb[0:1, :], in_=pvA[64:65, :])
        nc.vector.tensor_copy(out=zsb[64:65, :], in_=pvB[64:65, :])
        zi = att.tile([65, 512], F32, tag="zi", bufs=2, name=f"zi{hp}")
        with nc.allow_low_precision(reason="1/Z in bf16, tol 2e-2"):
            nc.vector.reciprocal_approx_fast(out=zi, in_=zsb)   # junk rows harmless

        nc.sync.dma_start(out=zscr2[2 * hp:2 * hp + 1, :], in_=zi[0:1, :])
        nc.sync.dma_start(out=zscr2[2 * hp + 1:2 * hp + 2, :], in_=zi[64:65, :])

        if hp == 5:
            # out-proj weights, row-major: woR[ct] = wo[ct*128:(ct+1)*128, :]
            for ct in range(8):
                wt = wopool.tile([128, DQ], BF16, tag="wo", name=f"woR{ct}")
                nc.sync.dma_start(out=wt, in_=wo[ct * 128:(ct + 1) * 128, :])
                woR.append(wt)
        if hp == 7:
            # P1: first half of out-proj (ct 0-3) as PE filler; fold in residual
            yac = att.tile([128, 2, 4, 512], BF16, tag="yac", bufs=1, name="yac")
            for c in range(2):
                for it in range(4):
                    po = ps.tile([128, 512], F32, tag="mm", name="po1")
                    for ct in range(4):
                        nc.tensor.matmul(po, valsT[:, ct, it * 128:(it + 1) * 128],
                                         woR[ct][:, c * 512:(c + 1) * 512],
                                         start=(ct == 0), stop=(ct == 3))
                    with nc.allow_low_precision(reason="partial out-proj sum bf16"):
                        nc.vector.tensor_add(yac[:, c, it, :], po,
                                             xp_sb[:, it, c * 512:(c + 1) * 512])

    # ---- P2: second half of out-proj (ct 4-7) + store ----
    normalize(7)
    for c in range(2):
        for it in range(4):
            po = ps.tile([128, 512], F32, tag="lg", name="po2")
            for ct in range(4, 8):
                nc.tensor.matmul(po, valsT[:, ct, it * 128:(it + 1) * 128],
                                 woR[ct][:, c * 512:(c + 1) * 512],
                                 start=(ct == 4), stop=(ct == 7))
            yt = att.tile([128, 512], F32, tag="yt", bufs=2, name="yt")
            nc.vector.tensor_add(yt, po, yac[:, c, it, :])
            nc.sync.dma_start(out=y.rearrange("(t p) d -> p t d", p=128)[:, it, c * 512:(c + 1) * 512],
                              in_=yt)
    att.release()
    e1.release()


def build_nc():
    nc = bacc_mod.Bacc()
    consts = nc.dram_tensor("consts", [128, 128], BF16, kind="ExternalInput")
    xp = nc.dram_tensor("xp", [NQS, DQ], F32, kind="ExternalInput")
    ctxt_t = nc.dram_tensor("ctxt_t", [DC, NCTX], BF16, kind="ExternalInput")
    wq = nc.dram_tensor("wq", [DQ, INNER], BF16, kind="ExternalInput")
    qb = nc.dram_tensor("qb", [1, INNER], F32, kind="ExternalInput")
    wk = nc.dram_tensor("wk", [DC, INNER], BF16, kind="ExternalInput")
    wv = nc.dram_tensor("wv", [DC, INNER], BF16, kind="ExternalInput")
    wo = nc.dram_tensor("wo", [INNER, DQ], BF16, kind="ExternalInput")
    y = nc.dram_tensor("y", [NQS, DQ], F32, kind="ExternalOutput")
    with ExitStack() as ctx:
        tc = ctx.enter_context(tile.TileContext(nc))
        _body(ctx, tc, nc, consts, xp, ctxt_t, wq, qb, wk, wv, wo, y)
    nc.compile()
    return nc


def make_in_maps(x, context, Wq, Wk, Wv, Wo, bo, gamma, beta):
    x = np.asarray(x, np.float32)
    context = np.asarray(context, np.float32)
    Wq = np.asarray(Wq, np.float32)
    Wk = np.asarray(Wk, np.float32)
    Wv = np.asarray(Wv, np.float32)
    Wo = np.asarray(Wo, np.float32)
    bo = np.asarray(bo, np.float32)
    gamma = np.asarray(gamma, np.float32)
    beta = np.asarray(beta, np.float32)

    wq_f = np.ascontiguousarray((gamma[:, None] * Wq).astype(ml_dtypes.bfloat16))
    qb_f = np.ascontiguousarray((beta @ Wq)[None, :])
    wv_bf = np.ascontiguousarray((Wv * np.float32(0.125)).astype(ml_dtypes.bfloat16))
    wk_bf = np.ascontiguousarray(Wk.astype(ml_dtypes.bfloat16))
    wo_bf = np.ascontiguousarray(Wo.astype(ml_dtypes.bfloat16))
    xp_full = x + bo  # residual with bo folded in

    consts = np.eye(128).astype(ml_dtypes.bfloat16)
    in_maps = []
    for c in range(N_CORES):
        b, qh = divmod(c, 2)
        in_maps.append({
            "consts": consts,
            "xp": np.ascontiguousarray(xp_full[b, qh * NQS:(qh + 1) * NQS, :]),
            "ctxt_t": np.ascontiguousarray(context[b].T.astype(ml_dtypes.bfloat16)),
            "wq": wq_f, "qb": qb_f, "wk": wk_bf, "wv": wv_bf,
            "wo": wo_bf,
        })
    return in_maps


_NC_CACHE = []


def kernel(x, context, Wq, Wk, Wv, Wo, bo, gamma, beta):
    from concourse.bass_utils import run_bass_kernel_spmd
    if not _NC_CACHE:
        _NC_CACHE.append(build_nc())
    nc = _NC_CACHE[0]
    in_maps = make_in_maps(x, context, Wq, Wk, Wv, Wo, bo, gamma, beta)
    res = run_bass_kernel_spmd(nc, in_maps, list(range(N_CORES)))
    y = np.empty((B, NQ, DQ), np.float32)
    for c in range(N_CORES):
        b, qh = divmod(c, 2)
        y[b, qh * NQS:(qh + 1) * NQS, :] = res.results[c]["y"]
    return y


# revision 20
# speedup vs baseline: 1.9480x; 1.0343x over previous
"""Cross-attention (LayerNorm -> MHA cross-attn -> out-proj -> residual) on 8 trn2 cores.

Sharding: core c -> (batch b = c//2, query-half qh = c%2). Each core computes all 16
heads for its 512 queries against the full 1024-token context of its batch. No
collectives needed; output shards are disjoint row blocks.

Host-side exact refactoring (all linear, fp32):
  - gamma folded into Wq:  Wq' = gamma[:,None] * Wq ;  qb = beta @ Wq
  - post-softmax scale 1/8 folded into Wv (power of two -> exact)
  - bo folded into the residual input: x' = x + bo
  - context shipped PRE-TRANSPOSED (ctxT = ctx.T, bf16)
  - wk/wv/wo shipped bf16 (psum accumulation stays fp32)

Device math per core:
  hn   = (x - mu) * rsqrt(var+eps)            (LayerNorm without affine)
  Q^T  = Wq'^T hn^T + qb
  K^T  = Wk^T ctx^T ;  V'' = [ctx Wv' | 1] with a ones column per head
  lT   = K^T_h (slice) x Q^T_h  per head      (logits, transposed layout [j, i])
  aT   = exp(lT)
  pv   = V''_h^T-contracted aT -> rows 0:64 = unnormalized vals, row 64 = Z
  vT   = valsU * (1/Z)  (recip on DVE per head-pair, junk rows harmless)
  y    = x' + vals @ Wo  (split: ct 0-3 accumulated during attention as PE filler)

v3: z-matmuls folded into pv via the ones column (-128 matmuls); out-proj split
P1/P2 so the tail stays PE-dense; weights bf16 so all weight DMAs issue at t=0.
"""

import numpy as np
import ml_dtypes
from contextlib import ExitStack

import concourse.bass as bass
import concourse.bacc as bacc_mod
import concourse.tile as tile
from concourse import mybir

F32 = mybir.dt.float32
F32R = mybir.dt.float32r
BF16 = mybir.dt.bfloat16
AF = mybir.ActivationFunctionType
ALU = mybir.AluOpType

B, NQ, NCTX, DQ, DC = 4, 1024, 1024, 1024, 768
H, DH, INNER = 16, 64, 1024
NQS = NQ // 2          # queries per core
EPS = 1e-5
N_CORES = 8


def _body(ctx, tc, nc, consts, xp, ctxt_t, wq, qb, wk, wv, wo, y):
    pers = ctx.enter_context(tc.tile_pool(name="pers", bufs=1))
    wpool = ctx.enter_context(tc.tile_pool(name="wpool", bufs=12))
    wqpool = ctx.enter_context(tc.tile_pool(name="wqpool", bufs=8))
    wopool = ctx.enter_context(tc.tile_pool(name="wopool", bufs=8))
    stat = ctx.enter_context(tc.tile_pool(name="stat", bufs=4))
    ps = ctx.enter_context(tc.tile_pool(name="ps", bufs=2, space="PSUM"))

    # ---- constants (identity shipped from host: f32r memset is invalid ISA) ----
    ident = pers.tile([128, 128], BF16, name="ident")
    nc.sync.dma_start(out=ident, in_=consts[0:128, :])
    eps_t = pers.tile([128, 1], F32, name="eps_t")
    nc.vector.memset(eps_t, EPS)
    zero_t = pers.tile([128, 1], F32, name="zero_t")
    nc.vector.memset(zero_t, 0.0)
    qb_sb = pers.tile([128, 8], F32, name="qb_sb")
    nc.sync.dma_start(out=qb_sb, in_=qb[0, :].rearrange("(m p) -> p m", p=128))

    # ---- persistent activations ----
    QT = pers.tile([128, 8, NQS], BF16, name="QT")          # Q^T  [inner, i]
    KT = pers.tile([128, 8, NCTX], BF16, name="KT")         # K^T  [inner, j]
    V_aug = pers.tile([128, 8, 16 * 65], BF16, name="V_aug")  # per head: [V_h | 1]
    nc.vector.memset(V_aug, 1.0)                            # ones cols stay 1.0
    valsT = pers.tile([128, 8, NQS], BF16, name="valsT")    # vals^T [inner, i]
    valsU = pers.tile([128, 8, NQS], BF16, name="valsU")    # unnormalized vals^T
    zscr2 = nc.dram_tensor("zscr2", [16, NQS], F32)         # 1/Z rows

    e1 = tc.alloc_tile_pool(name="e1", bufs=1)
    hT = e1.tile([128, 8, NQS], BF16, name="hT")            # hn^T  [d, i]
    e2 = tc.alloc_tile_pool(name="e2", bufs=1)
    ctxT = e2.tile([128, 6, NCTX], BF16, name="ctxT")       # ctx^T [d, j]
    for k in range(6):
        nc.sync.dma_start(out=ctxT[:, k, :], in_=ctxt_t[k * 128:(k + 1) * 128, :])
    xp_sb = pers.tile([128, 4, DQ], F32, name="xp_sb")
    nc.sync.dma_start(out=xp_sb, in_=xp.rearrange("(t p) d -> p t d", p=128))

    # ---- all projection weights DMA'd up front (bf16 halves the bytes) ----
    wv_t, wk_t, wq_t = [], [], []
    for k in range(6):
        wt = wpool.tile([128, INNER], BF16, tag="w", name=f"wv{k}")
        nc.sync.dma_start(out=wt, in_=wv[k * 128:(k + 1) * 128, :])
        wv_t.append(wt)
    for k in range(6):
        wt = wpool.tile([128, INNER], BF16, tag="w", name=f"wk{k}")
        nc.sync.dma_start(out=wt, in_=wk[k * 128:(k + 1) * 128, :])
        wk_t.append(wt)
    for k in range(8):
        wt = wqpool.tile([128, INNER], BF16, tag="wq", name=f"wq{k}")
        nc.sync.dma_start(out=wt, in_=wq[k * 128:(k + 1) * 128, :])
        wq_t.append(wt)

    # ---- LayerNorm (DVE only, before projections so hn is ready early) ----
    hn_t = []
    for it in range(4):
        st = stat.tile([128, 2, 6], F32, tag="st", name="st")
        for sb in range(2):
            nc.vector.bn_stats(out=st[:, sb, :], in_=xp_sb[:, it, sb * 512:(sb + 1) * 512])
        mv = stat.tile([128, 2], F32, tag="mv", name="mv")
        nc.vector.bn_aggr(out=mv, in_=st)
        sd = stat.tile([128, 1], F32, tag="sd", name="sd")
        nc.scalar.activation(out=sd, in_=mv[:, 1:2], func=AF.Sqrt, bias=eps_t, scale=1.0)
        rstd = stat.tile([128, 1], F32, tag="rstd", name="rstd")
        nc.vector.reciprocal(out=rstd, in_=sd)
        nmu = stat.tile([128, 1], F32, tag="nmu", name="nmu")
        nc.vector.tensor_scalar(out=nmu, in0=mv[:, 0:1], scalar1=-1.0, scalar2=None, op0=ALU.mult)
        hn = stat.tile([128, DQ], BF16, tag="hn", bufs=4, name="hn")
        nc.vector.tensor_scalar(out=hn, in0=xp_sb[:, it, :], scalar1=nmu, scalar2=rstd,
                                op0=ALU.add, op1=ALU.mult)
        hn_t.append(hn)

    # ---- V' = ctx @ Wv', evacuated into the 65-col-per-head augmented layout ----
    for c in range(2):
        for jt in range(8):
            pv = ps.tile([128, 512], F32, tag="mm", name="pv")
            for k in range(6):
                nc.tensor.matmul(pv, ctxT[:, k, jt * 128:(jt + 1) * 128],
                                 wv_t[k][:, c * 512:(c + 1) * 512],
                                 start=(k == 0), stop=(k == 5))
            nc.vector.tensor_copy(
                out=V_aug[:, jt, c * 520:(c + 1) * 520]
                    .rearrange("p (h e) -> p h e", h=8)[:, :, 0:64],
                in_=pv.rearrange("p (h e) -> p h e", h=8),
            )

    # ---- K^T = Wk^T @ ctx^T ----
    for m in range(8):
        for c in range(2):
            pk = ps.tile([128, 512], F32, tag="mm", name="pk")
            for k in range(6):
                nc.tensor.matmul(pk, wk_t[k][:, m * 128:(m + 1) * 128],
                                 ctxT[:, k, c * 512:(c + 1) * 512],
                                 start=(k == 0), stop=(k == 5))
            nc.scalar.activation(out=KT[:, m, c * 512:(c + 1) * 512], in_=pk,
                                 func=AF.Copy, bias=0.0)
    e2.release()
    # ---- transpose hn (PE, right after projections; hT copies on ACT) ----
    for it in range(4):
        for g in range(2):
            ptp = ps.tile([128, 4, 128], BF16, tag="mm", name="ptph")
            for q in range(4):
                dt_ = g * 4 + q
                nc.tensor.transpose(ptp[:, q, :], hn_t[it][:, dt_ * 128:(dt_ + 1) * 128], ident)
            nc.scalar.activation(
                out=hT[:, g * 4:(g + 1) * 4, it * 128:(it + 1) * 128],
                in_=ptp, func=AF.Copy, bias=0.0)

    att = tc.alloc_tile_pool(name="att", bufs=4)
    yac = None
    woR = []

    def normalize(h):
        zbs = att.tile([128, 512], F32, tag="zbs", bufs=2, name=f"zbs{h}")
        for bb in range(2):
            src = zscr2[2 * h + bb:2 * h + bb + 1, :]
            nc.sync.dma_start(
                out=zbs[64 * bb:64 * (bb + 1), :],
                in_=bass.AP(tensor=src.tensor, offset=src.offset, ap=[[0, 64], [1, 512]]),
            )
        with nc.allow_low_precision(reason="softmax normalize in bf16, tol 2e-2"):
            nc.vector.tensor_mul(valsT[:, h, :], valsU[:, h, :], zbs)

    # ---- attention, head pairs; Q-projection interleaved to keep PE dense ----
    for hp in range(8):
        hA, hB = 2 * hp, 2 * hp + 1
        if hp >= 1:
            normalize(hp - 1)
        pq = ps.tile([128, 512], F32, tag="mm", name="pq")
        for k in range(8):
            nc.tensor.matmul(pq, wq_t[k][:, hp * 128:(hp + 1) * 128], hT[:, k, :],
                             start=(k == 0), stop=(k == 7))
        with nc.allow_low_precision(reason="QT bf16, tol 2e-2"):
            nc.vector.tensor_scalar(out=QT[:, hp, :], in0=pq, scalar1=qb_sb[:, hp:hp + 1],
                                    scalar2=None, op0=ALU.add)
        aTA = att.tile([128, 8, NQS], BF16, tag="aT", bufs=3, name=f"aTA{hp}")
        aTB = att.tile([128, 8, NQS], BF16, tag="aT", bufs=3, name=f"aTB{hp}")
        for g in range(4):
            plA = ps.tile([128, 2, 512], F32, tag="lg", name="plA")
            plB = ps.tile([128, 2, 512], F32, tag="lg", name="plB")
            for bb in range(2):
                jt = g * 2 + bb
                nc.tensor.matmul(plA[:, bb, :], KT[0:64, hp, jt * 128:(jt + 1) * 128],
                                 QT[0:64, hp, :], start=True, stop=True,
                                 tile_position=(0, 0))
                nc.tensor.matmul(plB[:, bb, :], KT[64:128, hp, jt * 128:(jt + 1) * 128],
                                 QT[64:128, hp, :], start=True, stop=True,
                                 tile_position=(64, 0))
            nc.scalar.activation(out=aTA[:, g * 2:(g + 1) * 2, :], in_=plA, func=AF.Exp, bias=zero_t)
            nc.scalar.activation(out=aTB[:, g * 2:(g + 1) * 2, :], in_=plB, func=AF.Exp, bias=zero_t)
        pvA = ps.tile([65, 512], F32, tag="pva", bufs=1, name="pvA")
        pvB = ps.tile([65, 512], F32, tag="pvb", bufs=1, name="pvB")
        for jt in range(8):
            st_, sp = jt == 0, jt == 7
            nc.tensor.matmul(pvA, V_aug[:, jt, hA * 65:(hA + 1) * 65],
                             aTA[:, jt, :], start=st_, stop=sp)
            nc.tensor.matmul(pvB, V_aug[:, jt, hB * 65:(hB + 1) * 65],
                             aTB[:, jt, :], start=st_, stop=sp)
        # evacuate: vals rows 0:64, denominator row 64 (partition-aligned)
        nc.vector.tensor_copy(out=valsU[0:64, hp, :], in_=pvA[0:64, :])
        nc.vector.tensor_copy(out=valsU[64:128, hp, :], in_=pvB[0:64, :])
        zsb = att.tile([65, 512], F32, tag="zsb", bufs=2, name=f"zsb{hp}")
        nc.vector.tensor_copy(out=zsb[0:1, :], in_=pvA[64:65, :])
        nc.vector.tensor_copy(out=zsb[64:65, :], in_=pvB[64:65, :])
        zi = att.tile([65, 512], F32, tag="zi", bufs=2, name=f"zi{hp}")
        with nc.allow_low_precision(reason="1/Z in bf16, tol 2e-2"):
            nc.vector.reciprocal_approx_fast(out=zi, in_=zsb)   # junk rows harmless

        nc.sync.dma_start(out=zscr2[2 * hp:2 * hp + 1, :], in_=zi[0:1, :])
        nc.sync.dma_start(out=zscr2[2 * hp + 1:2 * hp + 2, :], in_=zi[64:65, :])

        if hp == 5:
            # out-proj weights, row-major: woR[ct] = wo[ct*128:(ct+1)*128, :]
            for ct in range(8):
                wt = wopool.tile([128, DQ], BF16, tag="wo", name=f"woR{ct}")
                nc.sync.dma_start(out=wt, in_=wo[ct * 128:(ct + 1) * 128, :])
                woR.append(wt)
        if hp == 7:
            # P1: first half of out-proj (ct 0-3) as PE filler; fold in residual
            yac = att.tile([128, 2, 4, 512], BF16, tag="yac", bufs=1, name="yac")
            for c in range(2):
                for it in range(4):
                    po = ps.tile([128, 512], F32, tag="mm", name="po1")
                    for ct in range(4):
                        nc.tensor.matmul(po, valsT[:, ct, it * 128:(it + 1) * 128],
                                         woR[ct][:, c * 512:(c + 1) * 512],
                                         start=(ct == 0), stop=(ct == 3))
                    with nc.allow_low_precision(reason="partial out-proj sum bf16"):
                        nc.vector.tensor_add(yac[:, c, it, :], po,
                                             xp_sb[:, it, c * 512:(c + 1) * 512])

    # ---- P2: second half of out-proj (ct 4-7) + store ----
    normalize(7)
    for c in range(2):
        for it in range(4):
            po = ps.tile([128, 512], F32, tag="lg", name="po2")
            for ct in range(4, 8):
                nc.tensor.matmul(po, valsT[:, ct, it * 128:(it + 1) * 128],
                                 woR[ct][:, c * 512:(c + 1) * 512],
                                 start=(ct == 4), stop=(ct == 7))
            yt = att.tile([128, 512], F32, tag="yt", bufs=6, name="yt")
            nc.vector.tensor_add(yt, po, yac[:, c, it, :])
            qeng = nc.sync if (c * 4 + it) % 2 == 0 else nc.scalar
            qeng.dma_start(out=y.rearrange("(t p) d -> p t d", p=128)[:, it, c * 512:(c + 1) * 512],
                           in_=yt)
    att.release()
    e1.release()


def build_nc():
    nc = bacc_mod.Bacc()
    consts = nc.dram_tensor("consts", [128, 128], BF16, kind="ExternalInput")
    xp = nc.dram_tensor("xp", [NQS, DQ], F32, kind="ExternalInput")
    ctxt_t = nc.dram_tensor("ctxt_t", [DC, NCTX], BF16, kind="ExternalInput")
    wq = nc.dram_tensor("wq", [DQ, INNER], BF16, kind="ExternalInput")
    qb = nc.dram_tensor("qb", [1, INNER], F32, kind="ExternalInput")
    wk = nc.dram_tensor("wk", [DC, INNER], BF16, kind="ExternalInput")
    wv = nc.dram_tensor("wv", [DC, INNER], BF16, kind="ExternalInput")
    wo = nc.dram_tensor("wo", [INNER, DQ], BF16, kind="ExternalInput")
    y = nc.dram_tensor("y", [NQS, DQ], F32, kind="ExternalOutput")
    with ExitStack() as ctx:
        tc = ctx.enter_context(tile.TileContext(nc))
        _body(ctx, tc, nc, consts, xp, ctxt_t, wq, qb, wk, wv, wo, y)
    nc.compile()
    return nc


def make_in_maps(x, context, Wq, Wk, Wv, Wo, bo, gamma, beta):
    x = np.asarray(x, np.float32)
    context = np.asarray(context, np.float32)
    Wq = np.asarray(Wq, np.float32)
    Wk = np.asarray(Wk, np.float32)
    Wv = np.asarray(Wv, np.float32)
    Wo = np.asarray(Wo, np.float32)
    bo = np.asarray(bo, np.float32)
    gamma = np.asarray(gamma, np.float32)
    beta = np.asarray(beta, np.float32)

    wq_f = np.ascontiguousarray((gamma[:, None] * Wq).astype(ml_dtypes.bfloat16))
    qb_f = np.ascontiguousarray((beta @ Wq)[None, :])
    wv_bf = np.ascontiguousarray((Wv * np.float32(0.125)).astype(ml_dtypes.bfloat16))
    wk_bf = np.ascontiguousarray(Wk.astype(ml_dtypes.bfloat16))
    wo_bf = np.ascontiguousarray(Wo.astype(ml_dtypes.bfloat16))
    xp_full = x + bo  # residual with bo folded in

    consts = np.eye(128).astype(ml_dtypes.bfloat16)
    in_maps = []
    for c in range(N_CORES):
        b, qh = divmod(c, 2)
        in_maps.append({
            "consts": consts,
            "xp": np.ascontiguousarray(xp_full[b, qh * NQS:(qh + 1) * NQS, :]),
            "ctxt_t": np.ascontiguousarray(context[b].T.astype(ml_dtypes.bfloat16)),
            "wq": wq_f, "qb": qb_f, "wk": wk_bf, "wv": wv_bf,
            "wo": wo_bf,
        })
    return in_maps


_NC_CACHE = []


def kernel(x, context, Wq, Wk, Wv, Wo, bo, gamma, beta):
    from concourse.bass_utils import run_bass_kernel_spmd
    if not _NC_CACHE:
        _NC_CACHE.append(build_nc())
    nc = _NC_CACHE[0]
    in_maps = make_in_maps(x, context, Wq, Wk, Wv, Wo, bo, gamma, beta)
    res = run_bass_kernel_spmd(nc, in_maps, list(range(N_CORES)))
    y = np.empty((B, NQ, DQ), np.float32)
    for c in range(N_CORES):
        b, qh = divmod(c, 2)
        y[b, qh * NQS:(qh + 1) * NQS, :] = res.results[c]["y"]
    return y
